# revision 12
# baseline (speedup 1.0000x reference)
"""Bass/Tile TRN2 kernel for multi-head self-attention with relative position bias.

Problem: B=4, T=2048, DIM=1024, HEADS=16, DH=64, causal + rel-pos-bias softmax.

Sharding (8 cores): data-parallel over batch (4) x tensor-parallel over heads (2x8).
Host sums the two head-group partials per batch.

Key structure (vs the naive 3-stage version):
 - stage 2 processes head PAIRS with 2-bank score tiles [128, (2 heads), 512]:
   one exp (ACT) and one bias-mult (DVE) instruction covers both heads,
   halving per-instruction fixed overheads.
 - software-pipelined emission: AV matmuls for j-block j are emitted KLA
   steps after their QK matmuls, so the in-order PE queue never stalls
   behind the exp->mult chain.
 - exact diagonal trimming: matmul/exp/mult widths shrink to the causal
   hull of each j-block (~10% less stage-2 work).
 - normalize path: reciprocal_approx_fast (DVE) -> gpsimd partition_broadcast
   (no DRAM bounce) -> deferred per-head multiply one unit later so the
   broadcast latency is hidden.
 - all 8 bias tables + W0 are DMA'd during stage 1.
 - stage 3 assembles [128, 1024] rows and issues one DMA per row-block,
   alternating SP/ACT queues.
 - stage 1 (qkv projection) is fused into stage 2: its matmul groups are
   interleaved between attention pipeline steps so the PE stream stays fed
   while ACT/DVE work on exp/bias of earlier i-chunks.
 - stage-3 chunks 0-2 are slotted into the last attention phase (ic=3),
   where the stage-1 psum ring is idle; only chunk 3 remains as tail.
 - bias-table/W0 DMAs are queued AFTER the stage-1 weights so the next
   For_i iteration's first matmul group is not gated on 7MB of DMA.
"""

import os

import numpy as np

import concourse.bass as bass
import concourse.tile as tile
from concourse import bacc, mybir
from concourse.bass_utils import run_bass_kernel_spmd

B, T, DIM, HEADS, DH = 4, 2048, 1024, 16, 64
N_CORES = 8
HPC = HEADS // 2          # heads per core = 8
FQ = HPC * DH             # per-core q/k/v feature width = 512
BH_C = 2432               # bias matrix free size  (max shift 1920 + 512)

F32 = mybir.dt.float32
F16 = mybir.dt.float16
DTM = F16
EXP = mybir.ActivationFunctionType.Exp
COPY = mybir.ActivationFunctionType.Copy

_CACHE = {}


def build_nc():
    nc = bacc.Bacc("TRN2", target_bir_lowering=False, debug=False,
                   enable_asserts=True, num_devices=N_CORES)
    xT_d = nc.dram_tensor("xT", [DIM, T], DTM, kind="ExternalInput").ap()
    wq_d = nc.dram_tensor("wq", [DIM, FQ], DTM, kind="ExternalInput").ap()
    wk_d = nc.dram_tensor("wk", [DIM, FQ], DTM, kind="ExternalInput").ap()
    wv_d = nc.dram_tensor("wv", [DIM, FQ], DTM, kind="ExternalInput").ap()
    w0_d = nc.dram_tensor("w0", [FQ, DIM], DTM, kind="ExternalInput").ap()
    bh_d = nc.dram_tensor("bh", [HPC, 128, BH_C], DTM, kind="ExternalInput").ap()
    id_d = nc.dram_tensor("identd", [128, 128], DTM, kind="ExternalInput").ap()
    on_d = nc.dram_tensor("onesd", [16, HPC], DTM, kind="ExternalInput").ap()
    out_d = nc.dram_tensor("out", [T, DIM], F32, kind="ExternalOutput").ap()

    krepeat = int(os.environ.get("KREPEAT", "1"))
    kskip = set(os.environ.get("KSKIP", "").split(","))  # timing-only: s1,s2,s3
    kprobe = set(os.environ.get("KPROBE", "").split(","))
    LA = int(os.environ.get("KLA", "2"))                 # AV lookahead steps
    with tile.TileContext(nc) as tc:
      import contextlib
      loop_cm = tc.For_i(0, krepeat, 1) if krepeat > 1 else contextlib.nullcontext()
      with loop_cm:
        with tc.tile_pool(name="persist", bufs=1) as persist:
            # persistent activations
            qT = persist.tile([128, 4, T], DTM)       # q^T: [(h%2)*64+d, h//2, t]
            kT = persist.tile([128, 4, T], DTM)
            v_sb = persist.tile([128, 16, HPC, DH + 1], DTM)  # [t%128, t//128, h, d|1]
            aoT = persist.tile([128, 4, T], DTM)      # attn-out^T, rows f=h*64+d
            bh_sb = persist.tile([128, HPC, BH_C], DTM)
            w0_sb = persist.tile([128, 4, DIM], DTM)

            nc.vector.memset(v_sb[:, :, :, DH], 1.0)  # ones column of v
            # prewarm the ACT exp table set (~2.7us) while stage 1 runs on PE
            warm = persist.tile([1, 2], F32)
            nc.scalar.activation(warm, warm, EXP)

            # ------- stages 1+2 fused: qkv projection || attention -------
            if "s1" in kskip:
                nc.vector.memset(qT, 0.0)
                nc.vector.memset(kT, 0.0)
                nc.vector.memset(v_sb[:, :, :, 0:DH], 0.0)
            if "s2" in kskip:
                nc.vector.memset(aoT, 0.0)
            with tc.tile_pool(name="wpool", bufs=1) as wpool, \
                 tc.tile_pool(name="xpool", bufs=16) as xpool, \
                 tc.tile_pool(name="ep", bufs=LA + 3) as ep, \
                 tc.tile_pool(name="rp", bufs=4) as rp, \
                 tc.tile_pool(name="rbp", bufs=4) as rbp, \
                 tc.tile_pool(name="stgp", bufs=2) as stgp, \
                 tc.tile_pool(name="ps1", bufs=1, space="PSUM") as ps1, \
                 tc.tile_pool(name="psS", bufs=2, space="PSUM") as psS, \
                 tc.tile_pool(name="psO", bufs=3, space="PSUM") as psO:
                wq_sb = wpool.tile([128, 8, FQ], DTM)
                wk_sb = wpool.tile([128, 8, FQ], DTM)
                wv_sb = wpool.tile([128, 8, FQ], DTM)
                if "s1" not in kskip:
                    for kd in range(8):
                        nc.scalar.dma_start(wq_sb[:, kd, :],
                                            wq_d[kd * 128:(kd + 1) * 128, :])
                        nc.scalar.dma_start(wk_sb[:, kd, :],
                                            wk_d[kd * 128:(kd + 1) * 128, :])
                    for kd in range(8):
                        nc.scalar.dma_start(wv_sb[:, kd, :],
                                            wv_d[kd * 128:(kd + 1) * 128, :])
                # bias tables + w0 AFTER the stage-1 weights on the ACT queue:
                # they are needed ~100us later, and putting them first would
                # gate the next iteration's first matmul group on 7MB of DMA.
                if "s2" not in kskip:
                    for h in range(HPC):
                        nc.scalar.dma_start(bh_sb[:, h, :], bh_d[h])
                if "s3" not in kskip:
                    for kf in range(4):
                        nc.scalar.dma_start(w0_sb[:, kf, :],
                                            w0_d[kf * 128:(kf + 1) * 128, :])

                def s1_chunk(tci):
                    """Stage-1 generator: yields after each matmul group."""
                    t0 = tci * 512
                    xts = []
                    for kd in range(8):
                        xt1 = xpool.tile([128, 512], DTM, name=f"xt{tci}_{kd}",
                                         tag="xt")
                        nc.sync.dma_start(
                            xt1, xT_d[kd * 128:(kd + 1) * 128, t0:t0 + 512])
                        xts.append(xt1)
                    yield
                    for mb in range(8):
                        w_sb, fb = (wq_sb, mb) if mb < 4 else (wk_sb, mb - 4)
                        ps = ps1.tile([128, 512], F32, name="psqk", tag="mm")
                        for kd in range(8):
                            nc.tensor.matmul(
                                ps, w_sb[:, kd, fb * 128:(fb + 1) * 128],
                                xts[kd], start=(kd == 0), stop=(kd == 7))
                        dst = qT if mb < 4 else kT
                        nc.vector.tensor_copy(dst[:, fb, t0:t0 + 512], ps)
                        yield
                    for tt in range(4):
                        ps = ps1.tile([128, 512], F32, name="psv", tag="mm")
                        for kd in range(8):
                            nc.tensor.matmul(
                                ps, xts[kd][:, tt * 128:(tt + 1) * 128],
                                wv_sb[:, kd, :], start=(kd == 0), stop=(kd == 7))
                        tb = tci * 4 + tt
                        nc.scalar.activation(
                            v_sb[:, tb, :, 0:DH],
                            ps.rearrange("p (h d) -> p h d", h=HPC), COPY)
                        yield

                norm_state = {"pending": None}

                def emit_norm_mults(st):
                    g, i0, po_e, po_o, rb_e, rb_o = st
                    nc.vector.tensor_tensor(
                        aoT[0:64, g, i0:i0 + 512], po_e[0:DH, :], rb_e,
                        mybir.AluOpType.mult)
                    stg = stgp.tile([64, 512], DTM, name="stg", tag="stg")
                    nc.vector.tensor_tensor(
                        stg, po_o[0:DH, :], rb_o, mybir.AluOpType.mult)
                    nc.sync.dma_start(aoT[64:128, g, i0:i0 + 512], stg)

                def s2_unit(g, ic):
                    """Stage-2 generator: yields after each pipeline step."""
                    i0 = ic * 512
                    nj = 4 * (ic + 1)
                    po_e = psO.tile([DH + 1, 512], F32, name="poe", tag="po")
                    po_o = psO.tile([DH + 1, 512], F32, name="poo", tag="po")
                    e_l, st_l = {}, {}
                    for step in range(nj + LA):
                        if step < nj:
                            jb = step
                            j0 = jb * 128
                            st = max(0, j0 - i0)
                            sc2 = psS.tile([128, 2, 512], F32, name="sc",
                                           tag="sc")
                            nc.tensor.matmul(
                                sc2[:, 0, st:512],
                                kT[0:64, g, j0:j0 + 128],
                                qT[0:64, g, i0 + st:i0 + 512],
                                start=True, stop=True)
                            nc.tensor.matmul(
                                sc2[:, 1, st:512],
                                kT[64:128, g, j0:j0 + 128],
                                qT[64:128, g, i0 + st:i0 + 512],
                                start=True, stop=True)
                            e2 = ep.tile([128, 2, 512], DTM, name="e2", tag="e")
                            c0 = i0 - j0 + 384
                            nc.scalar.activation(
                                e2[:, :, st:512], sc2[:, :, st:512], EXP)
                            nc.vector.tensor_tensor(
                                e2[:, :, st:512], e2[:, :, st:512],
                                bh_sb[:, 2 * g:2 * g + 2, c0 + st:c0 + 512],
                                mybir.AluOpType.mult)
                            e_l[jb] = e2
                            st_l[jb] = st
                        if step == 1 and norm_state["pending"] is not None:
                            emit_norm_mults(norm_state["pending"])
                            norm_state["pending"] = None
                        if step >= LA:
                            jb = step - LA
                            st = st_l[jb]
                            e2 = e_l.pop(jb)
                            nc.tensor.matmul(
                                po_e[:, st:512], v_sb[:, jb, 2 * g, :],
                                e2[:, 0, st:512],
                                start=(jb == 0), stop=(jb == nj - 1),
                                skip_group_check=True)
                            nc.tensor.matmul(
                                po_o[:, st:512], v_sb[:, jb, 2 * g + 1, :],
                                e2[:, 1, st:512],
                                start=(jb == 0), stop=(jb == nj - 1),
                                skip_group_check=True)
                        yield
                    if norm_state["pending"] is not None:  # nj==4, LA>=3 edge
                        emit_norm_mults(norm_state["pending"])
                        norm_state["pending"] = None
                    # normalize setup for this unit (mults deferred):
                    # reciprocal_approx_* reads the wrong partition for base!=0
                    # PSUM operands; copy the sums row to partition 0 first.
                    r_e = rp.tile([1, 512], F32, name="re", tag="r")
                    r_o = rp.tile([1, 512], F32, name="ro", tag="r")
                    s_e = rp.tile([1, 512], F32, name="se", tag="s")
                    s_o = rp.tile([1, 512], F32, name="so", tag="s")
                    nc.vector.tensor_copy(s_e, po_e[DH:DH + 1, :])
                    nc.vector.tensor_copy(s_o, po_o[DH:DH + 1, :])
                    nc.vector.reciprocal_approx_fast(r_e, s_e)
                    nc.vector.reciprocal_approx_fast(r_o, s_o)
                    rb_e = rbp.tile([64, 512], F32, name="rbe", tag="rb")
                    rb_o = rbp.tile([64, 512], F32, name="rbo", tag="rb")
                    nc.gpsimd.partition_broadcast(rb_e, r_e)
                    nc.gpsimd.partition_broadcast(rb_o, r_o)
                    norm_state["pending"] = (g, i0, po_e, po_o, rb_e, rb_o)

                def drain(gen):
                    for _ in gen:
                        pass

                def s2_phase(ic):
                    """All four head-pair units of i-chunk ic, sequentially."""
                    for g in range(4):
                        yield from s2_unit(g, ic)

                def s3_chunk(tci, osp):
                    """Stage-3 generator for the 4 t-blocks of chunk tci.
                    Requires the deferred norms of phase tci emitted (true
                    from phase 3 step >= 2 for tci <= 2)."""
                    for tt in range(4):
                        tb = tci * 4 + tt
                        o_sb = osp.tile([128, DIM], F32, name="osb", tag="osb")
                        for ncol in range(2):
                            n0 = ncol * 512
                            ps = ps1.tile([128, 512], F32, name="psf", tag="mm")
                            for kf in range(4):
                                nc.tensor.matmul(
                                    ps, aoT[:, kf, tb * 128:(tb + 1) * 128],
                                    w0_sb[:, kf, n0:n0 + 512],
                                    start=(kf == 0), stop=(kf == 3))
                            nc.scalar.activation(o_sb[:, n0:n0 + 512], ps, COPY)
                            yield
                        eng = nc.sync if tb % 2 == 0 else nc.scalar
                        eng.dma_start(out_d[tb * 128:(tb + 1) * 128, :], o_sb)

                def s3_chunks(tcis, osp):
                    for tci in tcis:
                        yield from s3_chunk(tci, osp)

                do_s1 = "s1" not in kskip
                do_s2 = "s2" not in kskip
                do_s3 = "s3" not in kskip
                with tc.tile_pool(name="osp", bufs=3) as osp:
                    if do_s1:
                        drain(s1_chunk(0))
                    for ic in range(4):
                        s1_gen = s1_chunk(ic + 1) if (ic < 3 and do_s1) else None
                        # phase 3 has no stage-1 work and an idle ps1 ring:
                        # slot stage-3 chunks 0-2 into it (their aoT windows
                        # are complete once phase 2's last norm clears, which
                        # happens at phase-3 unit g0 step 1).
                        s3_gen = (s3_chunks(range(3), osp)
                                  if (ic == 3 and do_s3 and do_s2
                                      and "nos3f" not in kprobe) else None)
                        s2_gen = s2_phase(ic) if do_s2 else None
                        first = True
                        while s1_gen is not None or s2_gen is not None \
                                or s3_gen is not None:
                            if s2_gen is not None:
                                for _ in range(2):
                                    try:
                                        next(s2_gen)
                                    except StopIteration:
                                        s2_gen = None
                                        break
                            if s1_gen is not None:
                                try:
                                    next(s1_gen)
                                except StopIteration:
                                    s1_gen = None
                            elif s3_gen is not None and not first:
                                try:
                                    next(s3_gen)
                                except StopIteration:
                                    s3_gen = None
                            first = False
                    if do_s2 and norm_state["pending"] is not None:
                        emit_norm_mults(norm_state["pending"])
                        norm_state["pending"] = None
                    if do_s3 and do_s2 and "nos3f" not in kprobe:
                        drain(s3_chunk(3, osp))
                    elif do_s3:
                        for tci in range(4):
                            drain(s3_chunk(tci, osp))
    nc.compile()
    return nc


def conv(a):
    return np.ascontiguousarray(a, dtype=np.float32).astype(np.float16)


def prep_inputs(x, W_qkv, W_0, rel_bias):
    """Shard + lay out the full inputs into 8 per-core input maps."""
    x = np.asarray(x, dtype=np.float32)
    W_qkv = np.asarray(W_qkv, dtype=np.float32)
    W_0 = np.asarray(W_0, dtype=np.float32)
    rel_bias = np.asarray(rel_bias, dtype=np.float32)

    # W_qkv columns are laid out (d, s, h): col = d*48 + s*16 + h
    wslab = W_qkv.reshape(DIM, DH, 3, HEADS)

    # bias matrices: bh_all[h, p, c] = exp(bias) | 0 at idx = p - c + 2431
    p = np.arange(128)[:, None]
    c = np.arange(BH_C)[None, :]
    idx = p - c + 2431                       # [128, C]
    safe = np.clip(idx, 0, 2 * T - 2)
    base = rel_bias[safe, :]                 # [128, C, HEADS]
    invalid = (idx < 0) | (idx > 2 * T - 2)
    masked = idx > T - 1                     # j > i  -> causal mask
    bh_all = np.where(masked[..., None] | invalid[..., None],
                      np.float32(-np.inf), base)
    bh_all = np.exp(np.transpose(bh_all, (2, 0, 1))).copy()  # [HEADS, 128, C]
    bh_all = conv(bh_all)

    identd = np.eye(128, dtype=np.float16)
    onesd = np.ones((16, HPC), dtype=np.float16)

    in_maps = []
    for core in range(N_CORES):
        b, hg = divmod(core, 2)
        h0 = hg * HPC
        # per-core weight slices, feature order f = h*64 + d
        wq = wslab[:, :, 0, h0:h0 + HPC].transpose(0, 2, 1).reshape(DIM, FQ)
        wq = wq * np.float32(DH ** -0.5)
        wk = wslab[:, :, 1, h0:h0 + HPC].transpose(0, 2, 1).reshape(DIM, FQ)
        wv = wslab[:, :, 2, h0:h0 + HPC].transpose(0, 2, 1).reshape(DIM, FQ)
        in_maps.append({
            "xT": conv(x[b].T),
            "wq": conv(wq),
            "wk": conv(wk),
            "wv": conv(wv),
            "w0": conv(W_0[h0 * DH:(h0 + HPC) * DH, :]),
            "bh": np.ascontiguousarray(bh_all[h0:h0 + HPC]),
            "identd": identd,
            "onesd": onesd,
        })
    return in_maps


def kernel(x, W_qkv, W_0, rel_bias):
    if "nc" not in _CACHE:
        _CACHE["nc"] = build_nc()
    nc = _CACHE["nc"]
    in_maps = prep_inputs(x, W_qkv, W_0, rel_bias)
    res = run_bass_kernel_spmd(nc, in_maps, core_ids=list(range(N_CORES)))
    out = np.empty((B, T, DIM), dtype=np.float32)
    for b in range(B):
        out[b] = res.results[2 * b]["out"] + res.results[2 * b + 1]["out"]
    return out
